# revision 4
# baseline (speedup 1.0000x reference)
"""Bidirectional GRU (B=64, T=512, I=512, H=1024) on 8 trn2 NeuronCores.

Sharding: core c = dir*4 + q handles direction dir (0=fwd, 1=bwd) and batch
quarter q (16 rows). The backward direction runs the identical program on a
time-reversed input sequence; the host reverses its outputs.

On-device layout is "h.T-packed": [128 partitions = H position within a
128-chunk, free col = chunk_idx*16 + batch]. Gate GEMMs use W as the
stationary operand so outputs land directly in this layout; x-projections
are computed on the PE in bursts of TB time steps into a ring tile.

Per-call wall time is dominated by the tunnel transfers (~42 MB/s) and by a
fixed per-loop-iteration cost for every engine/DMA-queue the body engages.
Hence: all h state is bf16 (hout too; host casts to f32), weights arrive as
quarter-shards and are AllGathered on-device within each direction's
4-core group, the whole xt slab is preloaded into SBUF before the loop
(matmuls use a dynamic column offset), and the loop body's only DMA is the
per-block hout store.
"""

import os
import sys

import numpy as np
import ml_dtypes

try:  # concourse/bass normally comes from the container's site config
    import concourse.bass  # noqa: F401
except ImportError:  # pragma: no cover
    for _p in ("/opt/trn_rl_repo", "/root/.axon_site/_ro/trn_rl_repo"):
        if os.path.isdir(_p) and _p not in sys.path:
            sys.path.insert(0, _p)

B, I, H = 64, 512, 1024
T = int(os.environ.get("BIDGRU_T", "512"))
NCORES = 8
BL = 16            # batch rows per core
NKH = 8            # hidden contraction chunks (1024/128)
NM = 8             # output H chunks (1024/128)
NKI = 4            # input contraction chunks (512/128)
TB = int(os.environ.get("BIDGRU_TB", "32"))  # time steps per burst block
NTB = T // TB      # t-blocks
BCOL = TB * BL     # cols per burst slab
CH = 3 * NKH * NM * 128   # wh packed cols
CX = 3 * NKI * NM * 128   # wx packed cols
WAG = int(os.environ.get("BIDGRU_WAG", "1"))  # weight allgather on/off
LAST_EXEC_NS = None

BF16 = ml_dtypes.bfloat16

_BUILD_CACHE = {}


def build():
    """Build the Bass program once; returns nc."""
    if "nc" in _BUILD_CACHE:
        return _BUILD_CACHE["nc"]

    import concourse.tile as tile
    import concourse.mybir as mybir
    from concourse import bacc
    from concourse.bass import ds

    f32 = mybir.dt.float32
    bf16 = mybir.dt.bfloat16
    AF = mybir.ActivationFunctionType

    nc = bacc.Bacc("TRN2", target_bir_lowering=False, debug=False,
                   num_devices=NCORES)

    xt_d = nc.dram_tensor("xt", [I, NTB * BCOL], bf16, kind="ExternalInput")
    wrows = 32 if WAG else 128
    wh_d = nc.dram_tensor("wh", [wrows, CH], bf16, kind="ExternalInput")
    wx_d = nc.dram_tensor("wx", [wrows, CX], bf16, kind="ExternalInput")
    bias_d = nc.dram_tensor("bias", [128, 3 * NM], f32, kind="ExternalInput")
    h0_d = nc.dram_tensor("h0", [128, NKH * BL], f32, kind="ExternalInput")
    # hout row tb*128+p, col t*(NM*BL) + m*BL + b
    hout_d = nc.dram_tensor("hout", [NTB * 128, TB * NM * BL], bf16,
                            kind="ExternalOutput")

    xt = xt_d.ap()
    bias = bias_d.ap()
    h0 = h0_d.ap()
    hout = hout_d.ap()

    def whsl(g, k, m):
        i = (g * NKH + k) * NM + m
        return slice(i * 128, (i + 1) * 128)

    def wxsl(g, k, m):
        i = (g * NKI + k) * NM + m
        return slice(i * 128, (i + 1) * 128)

    with tile.TileContext(nc) as tc:
        from contextlib import ExitStack
        ctx = ExitStack()
        with ctx:
            singles = ctx.enter_context(tc.tile_pool(name="singles", bufs=1))
            ring_pool = ctx.enter_context(tc.tile_pool(name="ringp", bufs=1))
            st_pool = ctx.enter_context(tc.tile_pool(name="stp", bufs=2))
            tmp = ctx.enter_context(tc.tile_pool(name="tmp", bufs=2))
            ps_burst = ctx.enter_context(
                tc.tile_pool(name="ps_burst", bufs=2, space="PSUM"))
            ps_step = ctx.enter_context(
                tc.tile_pool(name="ps_step", bufs=2, space="PSUM"))

            # on-device weight AllGather within each direction's 4-core
            # group: each core ships a 32-row shard, the group reassembles
            # the full 128-row packed weight matrices in DRAM
            if WAG:
                dram = ctx.enter_context(
                    tc.tile_pool(name="dram", bufs=1, space="DRAM"))
                whb = dram.tile([32, CH], bf16)
                wxb = dram.tile([32, CX], bf16)
                whg = dram.tile([128, CH], bf16)
                wxg = dram.tile([128, CX], bf16)
                nc.gpsimd.dma_start(whb[:], wh_d.ap()[:, :])
                nc.gpsimd.dma_start(wxb[:], wx_d.ap()[:, :])
                groups = [[0, 1, 2, 3], [4, 5, 6, 7]]
                nc.gpsimd.collective_compute(
                    "AllGather", mybir.AluOpType.bypass,
                    replica_groups=groups, ins=[whb.opt()], outs=[whg.opt()])
                nc.gpsimd.collective_compute(
                    "AllGather", mybir.AluOpType.bypass,
                    replica_groups=groups, ins=[wxb.opt()], outs=[wxg.opt()])
                wh, wx = whg[:], wxg[:]
            else:
                wh, wx = wh_d.ap(), wx_d.ap()

            wh_sb = singles.tile([128, CH], bf16)
            wx_sb = singles.tile([128, CX], bf16)
            bias_sb = singles.tile([128, 3 * NM], f32)
            h0_sb = singles.tile([128, NKH * BL], f32)
            hcar16 = singles.tile([128, NM * BL], bf16)
            xt_all = singles.tile([128, NKI, NTB * BCOL], bf16)

            # per-(g,k) chunk DMAs: keeps each load on a single DMA queue so
            # consumer matmuls wait on few semaphores (ISA wait-slot limit)
            for g in range(3):
                for k in range(NKH):
                    sl = slice(whsl(g, k, 0).start, whsl(g, k, NM - 1).stop)
                    nc.sync.dma_start(out=wh_sb[:, sl], in_=wh[:, sl])
                for k in range(NKI):
                    sl = slice(wxsl(g, k, 0).start, wxsl(g, k, NM - 1).stop)
                    nc.sync.dma_start(out=wx_sb[:, sl], in_=wx[:, sl])
            for k in range(NKI):
                nc.sync.dma_start(out=xt_all[:, k, :],
                                  in_=xt[k * 128:(k + 1) * 128, :])
            nc.sync.dma_start(out=bias_sb[:], in_=bias[:, :])
            nc.sync.dma_start(out=h0_sb[:], in_=h0[:, :])
            nc.vector.tensor_copy(out=hcar16[:], in_=h0_sb[:])

            def burst(xoff, ring):
                """x-projections (+bias) for the t-block at col `xoff`."""
                for g in range(3):
                    for m in range(NM):
                        ps = ps_burst.tile([128, BCOL], f32)
                        for k in range(NKI):
                            nc.tensor.matmul(
                                ps[:], wx_sb[:, wxsl(g, k, m)],
                                xt_all[:, k, ds(xoff, BCOL)],
                                start=(k == 0), stop=(k == NKI - 1))
                        c = g * NM + m
                        nc.vector.tensor_scalar_add(
                            ring[:, g, :, m * BL:(m + 1) * BL],
                            ps[:].rearrange("p (t b) -> p t b", b=BL),
                            bias_sb[:, c:c + 1])

            HF = NM * BL // 2    # 64-col half

            def step(ring, st16, j):
                h16_in = hcar16[:] if j == 0 else st16[:, j - 1, :]

                # R matmuls, k-inner: the first k-chunks only need the first
                # half of h16 (produced early by the previous step's
                # half-pipelined tail)
                ps_r = ps_step.tile([128, NM * BL], f32, tag="ps_r", bufs=1)
                for m in range(NM):
                    for k in range(NKH):
                        nc.tensor.matmul(
                            ps_r[:, m * BL:(m + 1) * BL],
                            wh_sb[:, whsl(0, k, m)],
                            h16_in[:, k * BL:(k + 1) * BL],
                            start=(k == 0), stop=(k == NKH - 1))
                pre_r = tmp.tile([128, NM * BL], f32, tag="pre_r")
                nc.vector.tensor_add(out=pre_r[:], in0=ps_r[:],
                                     in1=ring[:, 0, j, :])
                r_g = tmp.tile([128, NM * BL], f32, tag="r_g")
                nc.scalar.activation(out=r_g[:], in_=pre_r[:], func=AF.Sigmoid)

                # Z matmuls run on the PE while R's glue chain goes
                ps_z = ps_step.tile([128, NM * BL], f32, tag="ps_z", bufs=1)
                for m in range(NM):
                    for k in range(NKH):
                        nc.tensor.matmul(
                            ps_z[:, m * BL:(m + 1) * BL],
                            wh_sb[:, whsl(1, k, m)],
                            h16_in[:, k * BL:(k + 1) * BL],
                            start=(k == 0), stop=(k == NKH - 1))

                rh16 = tmp.tile([128, NM * BL], bf16, tag="rh16")
                nc.vector.tensor_mul(out=rh16[:], in0=r_g[:], in1=h16_in)
                pre_z = tmp.tile([128, NM * BL], f32, tag="pre_z")
                nc.vector.tensor_add(out=pre_z[:], in0=ps_z[:],
                                     in1=ring[:, 1, j, :])
                z_g = tmp.tile([128, NM * BL], f32, tag="z_g")
                nc.scalar.activation(out=z_g[:], in_=pre_z[:], func=AF.Sigmoid)

                # candidate matmuls in two half-tiles on DIFFERENT psum
                # banks: the tail can consume half 0 while the PE still
                # accumulates half 1
                ps_hh = [ps_step.tile([128, HF], f32, tag=f"ps_h{i}",
                                      name=f"ps_h{i}", bufs=2)
                         for i in range(2)]
                for m in range(NM):
                    ph = ps_hh[m // (NM // 2)]
                    mo = m % (NM // 2)
                    for k in range(NKH):
                        nc.tensor.matmul(
                            ph[:, mo * BL:(mo + 1) * BL],
                            wh_sb[:, whsl(2, k, m)],
                            rh16[:, k * BL:(k + 1) * BL],
                            start=(k == 0), stop=(k == NKH - 1))

                # tail, split into column halves so h16's first half is
                # ready while the second half of ps_h is still accumulating
                for hi in range(2):
                    cs = slice(hi * HF, (hi + 1) * HF)
                    pre_h = tmp.tile([128, HF], f32, tag=f"pre_h{hi}")
                    nc.vector.tensor_add(out=pre_h[:], in0=ps_hh[hi][:],
                                         in1=ring[:, 2, j, cs])
                    ht = tmp.tile([128, HF], f32, tag=f"ht{hi}")
                    nc.scalar.activation(out=ht[:], in_=pre_h[:],
                                         func=AF.Tanh)
                    d = tmp.tile([128, HF], f32, tag=f"d{hi}")
                    nc.vector.tensor_sub(out=d[:], in0=ht[:],
                                         in1=h16_in[:, cs])
                    e = tmp.tile([128, HF], f32, tag=f"e{hi}")
                    nc.vector.tensor_mul(out=e[:], in0=z_g[:, cs], in1=d[:])
                    nc.vector.tensor_add(out=st16[:, j, cs], in0=e[:],
                                         in1=h16_in[:, cs])

            with tc.For_i(0, NTB, 1,
                          hint_engines=(mybir.EngineType.PE,)) as iv:
                xoff = nc.snap(iv * BCOL)
                hoff = nc.snap(iv * 128)
                ring = ring_pool.tile([128, 3, TB, NM * BL], bf16)
                burst(xoff, ring)
                st16 = st_pool.tile([128, TB, NM * BL], bf16)
                for j in range(TB):
                    step(ring, st16, j)
                nc.vector.tensor_copy(out=hcar16[:], in_=st16[:, TB - 1, :])
                nc.sync.dma_start(
                    out=hout[ds(hoff, 128), :],
                    in_=st16[:].rearrange("p a b -> p (a b)"))

    nc.compile()
    _BUILD_CACHE["nc"] = nc
    return nc


def _pack_dir(inputs, d):
    """Pack one direction's weights/bias (shared by its 4 cores)."""
    sfx = "f" if d == 0 else "b"

    def pack_wh(w):
        return np.ascontiguousarray(
            np.asarray(w).reshape(NKH, 128, NM, 128)
            .transpose(1, 0, 2, 3)).reshape(128, NKH * NM * 128)

    def pack_wx(w):
        return np.ascontiguousarray(
            np.asarray(w).reshape(NKI, 128, NM, 128)
            .transpose(1, 0, 2, 3)).reshape(128, NKI * NM * 128)

    whp = np.concatenate(
        [pack_wh(inputs[f"W_h{g}_{sfx}"]) for g in ("r", "z", "h")],
        axis=1).astype(BF16)
    wxp = np.concatenate(
        [pack_wx(inputs[f"W_x{g}_{sfx}"]) for g in ("r", "z", "h")],
        axis=1).astype(BF16)
    biasp = np.ascontiguousarray(np.concatenate(
        [np.asarray(inputs[f"b_{g}_{sfx}"]).reshape(NM, 128).T
         for g in ("r", "z", "h")], axis=1)).astype(np.float32)
    return whp, wxp, biasp


def kernel(**inputs):
    global LAST_EXEC_NS
    from concourse.bass_utils import run_bass_kernel_spmd

    nc = build()

    # xTt[i, t, b] = x[b, t, i], bf16, one shuffle for all 8 cores
    xb = np.asarray(inputs["inputs"]).astype(BF16)
    xTt = np.ascontiguousarray(xb.transpose(2, 1, 0))
    dir_packs = [_pack_dir(inputs, d) for d in range(2)]

    in_maps = []
    for c in range(NCORES):
        d, q = c // 4, c % 4
        whp, wxp, biasp = dir_packs[d]
        if WAG:
            whp = whp[32 * q:32 * (q + 1), :]
            wxp = wxp[32 * q:32 * (q + 1), :]
        view = (xTt[:, :, q * BL:(q + 1) * BL] if d == 0
                else xTt[:, ::-1, q * BL:(q + 1) * BL])
        hp = np.asarray(
            inputs[f"h_prev_{'forward' if d == 0 else 'backward'}"])
        h0p = np.ascontiguousarray(
            hp[q * BL:(q + 1) * BL].T.reshape(NKH, 128, BL)
            .transpose(1, 0, 2)).reshape(128, NKH * BL).astype(np.float32)
        in_maps.append({
            "xt": np.ascontiguousarray(view).reshape(I, T * BL),
            "wh": np.ascontiguousarray(whp),
            "wx": np.ascontiguousarray(wxp),
            "bias": biasp, "h0": h0p,
        })

    trace = bool(int(os.environ.get("BIDGRU_TRACE", "0")))
    res = run_bass_kernel_spmd(nc, in_maps, core_ids=list(range(NCORES)),
                               trace=trace)
    if res.exec_time_ns:
        LAST_EXEC_NS = res.exec_time_ns

    out = np.empty((B, T, 2 * H), dtype=np.float32)
    for c in range(NCORES):
        d, q = c // 4, c % 4
        ho = res.results[c]["hout"].reshape(NTB, 128, TB, NM, BL)
        # ho[tb, p, t, m, b] = h[b, tb*TB+t, m*128+p]
        hv = ho.transpose(4, 0, 2, 3, 1).reshape(BL, T, H)
        if d == 1:
            hv = hv[:, ::-1, :]
        out[q * BL:(q + 1) * BL, :, d * H:(d + 1) * H] = hv
    return out


if __name__ == "__main__":
    sys.path.insert(0, "/root/problem")
    build()
    print("build ok")


# revision 5
# speedup vs baseline: 4.2873x; 4.2873x over previous
"""Bidirectional GRU (B=64, T=512, I=512, H=1024) on 8 trn2 NeuronCores.

Sharding: core c = dir*4 + q handles direction dir (0=fwd, 1=bwd) and batch
quarter q (16 rows). The backward direction runs the identical program on a
time-reversed input sequence; the host reverses its outputs.

On-device layout is "h.T-packed": [128 partitions = H position within a
128-chunk, free col = chunk_idx*16 + batch]. Gate GEMMs use W as the
stationary operand so outputs land directly in this layout; x-projections
are computed on the PE in bursts of TB time steps into a ring tile.

Per-call wall time is dominated by the tunnel transfers (~42 MB/s) and by a
fixed per-loop-iteration cost for every engine/DMA-queue the body engages.
Hence: all h state is bf16 (hout too; host casts to f32), weights arrive as
quarter-shards and are AllGathered on-device within each direction's
4-core group, the whole xt slab is preloaded into SBUF before the loop
(matmuls use a dynamic column offset), and the loop body's only DMA is the
per-block hout store.
"""

import os
import sys

import numpy as np
import ml_dtypes

try:  # concourse/bass normally comes from the container's site config
    import concourse.bass  # noqa: F401
except ImportError:  # pragma: no cover
    for _p in ("/opt/trn_rl_repo", "/root/.axon_site/_ro/trn_rl_repo"):
        if os.path.isdir(_p) and _p not in sys.path:
            sys.path.insert(0, _p)

B, I, H = 64, 512, 1024
T = int(os.environ.get("BIDGRU_T", "512"))
NCORES = 8
BL = 16            # batch rows per core
NKH = 8            # hidden contraction chunks (1024/128)
NM = 8             # output H chunks (1024/128)
NKI = 4            # input contraction chunks (512/128)
TB = int(os.environ.get("BIDGRU_TB", "32"))  # time steps per burst block
NTB = T // TB      # t-blocks
BCOL = TB * BL     # cols per burst slab
CH = 3 * NKH * NM * 128   # wh packed cols
CX = 3 * NKI * NM * 128   # wx packed cols
WAG = int(os.environ.get("BIDGRU_WAG", "1"))  # weight allgather on/off
LAST_EXEC_NS = None

BF16 = ml_dtypes.bfloat16

_BUILD_CACHE = {}


def build():
    """Build the Bass program once; returns nc."""
    if "nc" in _BUILD_CACHE:
        return _BUILD_CACHE["nc"]

    import concourse.tile as tile
    import concourse.mybir as mybir
    from concourse import bacc
    from concourse.bass import ds

    f32 = mybir.dt.float32
    bf16 = mybir.dt.bfloat16
    AF = mybir.ActivationFunctionType

    nc = bacc.Bacc("TRN2", target_bir_lowering=False, debug=False,
                   num_devices=NCORES)

    xt_d = nc.dram_tensor("xt", [I, NTB * BCOL], bf16, kind="ExternalInput")
    wrows = 32 if WAG else 128
    wh_d = nc.dram_tensor("wh", [wrows, CH], bf16, kind="ExternalInput")
    wx_d = nc.dram_tensor("wx", [wrows, CX], bf16, kind="ExternalInput")
    bias_d = nc.dram_tensor("bias", [128, 3 * NM], f32, kind="ExternalInput")
    h0_d = nc.dram_tensor("h0", [128, NKH * BL], f32, kind="ExternalInput")
    # hout row tb*128+p, col t*(NM*BL) + m*BL + b
    hout_d = nc.dram_tensor("hout", [NTB * 128, TB * NM * BL], bf16,
                            kind="ExternalOutput")

    xt = xt_d.ap()
    bias = bias_d.ap()
    h0 = h0_d.ap()
    hout = hout_d.ap()

    def whsl(g, k, m):
        i = (g * NKH + k) * NM + m
        return slice(i * 128, (i + 1) * 128)

    def wxsl(g, k, m):
        i = (g * NKI + k) * NM + m
        return slice(i * 128, (i + 1) * 128)

    with tile.TileContext(nc) as tc:
        from contextlib import ExitStack
        ctx = ExitStack()
        with ctx:
            singles = ctx.enter_context(tc.tile_pool(name="singles", bufs=1))
            xtb_pool = ctx.enter_context(tc.tile_pool(name="xtbp", bufs=2))
            ring_pool = ctx.enter_context(tc.tile_pool(name="ringp", bufs=1))
            st_pool = ctx.enter_context(tc.tile_pool(name="stp", bufs=2))
            tmp = ctx.enter_context(tc.tile_pool(name="tmp", bufs=2))
            ps_burst = ctx.enter_context(
                tc.tile_pool(name="ps_burst", bufs=2, space="PSUM"))
            ps_step = ctx.enter_context(
                tc.tile_pool(name="ps_step", bufs=2, space="PSUM"))

            # on-device weight AllGather within each direction's 4-core
            # group: each core ships a 32-row shard, the group reassembles
            # the full 128-row packed weight matrices in DRAM
            if WAG:
                dram = ctx.enter_context(
                    tc.tile_pool(name="dram", bufs=1, space="DRAM"))
                whb = dram.tile([32, CH], bf16)
                wxb = dram.tile([32, CX], bf16)
                whg = dram.tile([128, CH], bf16)
                wxg = dram.tile([128, CX], bf16)
                nc.gpsimd.dma_start(whb[:], wh_d.ap()[:, :])
                nc.gpsimd.dma_start(wxb[:], wx_d.ap()[:, :])
                groups = [[0, 1, 2, 3], [4, 5, 6, 7]]
                nc.gpsimd.collective_compute(
                    "AllGather", mybir.AluOpType.bypass,
                    replica_groups=groups, ins=[whb.opt()], outs=[whg.opt()])
                nc.gpsimd.collective_compute(
                    "AllGather", mybir.AluOpType.bypass,
                    replica_groups=groups, ins=[wxb.opt()], outs=[wxg.opt()])
                wh, wx = whg[:], wxg[:]
            else:
                wh, wx = wh_d.ap(), wx_d.ap()

            wh_sb = singles.tile([128, CH], bf16)
            wx_sb = singles.tile([128, CX], bf16)
            bias_sb = singles.tile([128, 3 * NM], f32)
            h0_sb = singles.tile([128, NKH * BL], f32)
            hcar16 = singles.tile([128, NM * BL], bf16)
            xt_all = singles.tile([128, NKI, NTB * BCOL], bf16)

            # per-(g,k) chunk DMAs: keeps each load on a single DMA queue so
            # consumer matmuls wait on few semaphores (ISA wait-slot limit)
            for g in range(3):
                for k in range(NKH):
                    sl = slice(whsl(g, k, 0).start, whsl(g, k, NM - 1).stop)
                    nc.sync.dma_start(out=wh_sb[:, sl], in_=wh[:, sl])
                for k in range(NKI):
                    sl = slice(wxsl(g, k, 0).start, wxsl(g, k, NM - 1).stop)
                    nc.sync.dma_start(out=wx_sb[:, sl], in_=wx[:, sl])
            for k in range(NKI):
                nc.sync.dma_start(out=xt_all[:, k, :],
                                  in_=xt[k * 128:(k + 1) * 128, :])
            nc.sync.dma_start(out=bias_sb[:], in_=bias[:, :])
            nc.sync.dma_start(out=h0_sb[:], in_=h0[:, :])
            nc.vector.tensor_copy(out=hcar16[:], in_=h0_sb[:])

            def burst(xoff, ring):
                """x-projections (+bias) for the t-block at col `xoff`."""
                for g in range(3):
                    for m in range(NM):
                        ps = ps_burst.tile([128, BCOL], f32)
                        for k in range(NKI):
                            nc.tensor.matmul(
                                ps[:], wx_sb[:, wxsl(g, k, m)],
                                xt_all[:, k, ds(xoff, BCOL)],
                                start=(k == 0), stop=(k == NKI - 1))
                        c = g * NM + m
                        nc.vector.tensor_scalar_add(
                            ring[:, g, :, m * BL:(m + 1) * BL],
                            ps[:].rearrange("p (t b) -> p t b", b=BL),
                            bias_sb[:, c:c + 1])

            HF = NM * BL // 2    # 64-col half

            def step(ring, st16, j):
                h16_in = hcar16[:] if j == 0 else st16[:, j - 1, :]

                # R matmuls, k-inner: the first k-chunks only need the first
                # half of h16 (produced early by the previous step's
                # half-pipelined tail)
                ps_r = ps_step.tile([128, NM * BL], f32, tag="ps_r", bufs=1)
                for m in range(NM):
                    for k in range(NKH):
                        nc.tensor.matmul(
                            ps_r[:, m * BL:(m + 1) * BL],
                            wh_sb[:, whsl(0, k, m)],
                            h16_in[:, k * BL:(k + 1) * BL],
                            start=(k == 0), stop=(k == NKH - 1))
                pre_r = tmp.tile([128, NM * BL], f32, tag="pre_r")
                nc.vector.tensor_add(out=pre_r[:], in0=ps_r[:],
                                     in1=ring[:, 0, j, :])
                r_g = tmp.tile([128, NM * BL], f32, tag="r_g")
                nc.scalar.activation(out=r_g[:], in_=pre_r[:], func=AF.Sigmoid)

                # Z matmuls run on the PE while R's glue chain goes
                ps_z = ps_step.tile([128, NM * BL], f32, tag="ps_z", bufs=1)
                for m in range(NM):
                    for k in range(NKH):
                        nc.tensor.matmul(
                            ps_z[:, m * BL:(m + 1) * BL],
                            wh_sb[:, whsl(1, k, m)],
                            h16_in[:, k * BL:(k + 1) * BL],
                            start=(k == 0), stop=(k == NKH - 1))

                rh16 = tmp.tile([128, NM * BL], bf16, tag="rh16")
                nc.vector.tensor_mul(out=rh16[:], in0=r_g[:], in1=h16_in)
                pre_z = tmp.tile([128, NM * BL], f32, tag="pre_z")
                nc.vector.tensor_add(out=pre_z[:], in0=ps_z[:],
                                     in1=ring[:, 1, j, :])
                z_g = tmp.tile([128, NM * BL], f32, tag="z_g")
                nc.scalar.activation(out=z_g[:], in_=pre_z[:], func=AF.Sigmoid)

                # candidate matmuls in two half-tiles on DIFFERENT psum
                # banks: the tail can consume half 0 while the PE still
                # accumulates half 1
                ps_hh = [ps_step.tile([128, HF], f32, tag=f"ps_h{i}",
                                      name=f"ps_h{i}", bufs=2)
                         for i in range(2)]
                for m in range(NM):
                    ph = ps_hh[m // (NM // 2)]
                    mo = m % (NM // 2)
                    for k in range(NKH):
                        nc.tensor.matmul(
                            ph[:, mo * BL:(mo + 1) * BL],
                            wh_sb[:, whsl(2, k, m)],
                            rh16[:, k * BL:(k + 1) * BL],
                            start=(k == 0), stop=(k == NKH - 1))

                # tail, split into column halves so h16's first half is
                # ready while the second half of ps_h is still accumulating
                for hi in range(2):
                    cs = slice(hi * HF, (hi + 1) * HF)
                    pre_h = tmp.tile([128, HF], f32, tag=f"pre_h{hi}")
                    nc.vector.tensor_add(out=pre_h[:], in0=ps_hh[hi][:],
                                         in1=ring[:, 2, j, cs])
                    ht = tmp.tile([128, HF], f32, tag=f"ht{hi}")
                    nc.scalar.activation(out=ht[:], in_=pre_h[:],
                                         func=AF.Tanh)
                    d = tmp.tile([128, HF], f32, tag=f"d{hi}")
                    nc.vector.tensor_sub(out=d[:], in0=ht[:],
                                         in1=h16_in[:, cs])
                    e = tmp.tile([128, HF], f32, tag=f"e{hi}")
                    nc.vector.tensor_mul(out=e[:], in0=z_g[:, cs], in1=d[:])
                    nc.vector.tensor_add(out=st16[:, j, cs], in0=e[:],
                                         in1=h16_in[:, cs])

            with tc.For_i(0, NTB, 1,
                          hint_engines=(mybir.EngineType.PE,)) as iv:
                xoff = nc.snap(iv * BCOL)
                hoff = nc.snap(iv * 128)
                ring = ring_pool.tile([128, 3, TB, NM * BL], bf16)
                burst(xoff, ring)
                st16 = st_pool.tile([128, TB, NM * BL], bf16)
                for j in range(TB):
                    step(ring, st16, j)
                nc.vector.tensor_copy(out=hcar16[:], in_=st16[:, TB - 1, :])
                nc.sync.dma_start(
                    out=hout[ds(hoff, 128), :],
                    in_=st16[:].rearrange("p a b -> p (a b)"))

    nc.compile()
    _BUILD_CACHE["nc"] = nc
    return nc


def _pack_dir(inputs, d):
    """Pack one direction's weights/bias (shared by its 4 cores)."""
    sfx = "f" if d == 0 else "b"

    def pack_wh(w):
        return np.ascontiguousarray(
            np.asarray(w).reshape(NKH, 128, NM, 128)
            .transpose(1, 0, 2, 3)).reshape(128, NKH * NM * 128)

    def pack_wx(w):
        return np.ascontiguousarray(
            np.asarray(w).reshape(NKI, 128, NM, 128)
            .transpose(1, 0, 2, 3)).reshape(128, NKI * NM * 128)

    whp = np.concatenate(
        [pack_wh(inputs[f"W_h{g}_{sfx}"]) for g in ("r", "z", "h")],
        axis=1).astype(BF16)
    wxp = np.concatenate(
        [pack_wx(inputs[f"W_x{g}_{sfx}"]) for g in ("r", "z", "h")],
        axis=1).astype(BF16)
    biasp = np.ascontiguousarray(np.concatenate(
        [np.asarray(inputs[f"b_{g}_{sfx}"]).reshape(NM, 128).T
         for g in ("r", "z", "h")], axis=1)).astype(np.float32)
    return whp, wxp, biasp


def kernel(**inputs):
    global LAST_EXEC_NS
    from concourse.bass_utils import run_bass_kernel_spmd

    nc = build()

    # xTt[i, t, b] = x[b, t, i], bf16, one shuffle for all 8 cores
    xb = np.asarray(inputs["inputs"]).astype(BF16)
    xTt = np.ascontiguousarray(xb.transpose(2, 1, 0))
    dir_packs = [_pack_dir(inputs, d) for d in range(2)]

    in_maps = []
    for c in range(NCORES):
        d, q = c // 4, c % 4
        whp, wxp, biasp = dir_packs[d]
        if WAG:
            whp = whp[32 * q:32 * (q + 1), :]
            wxp = wxp[32 * q:32 * (q + 1), :]
        view = (xTt[:, :, q * BL:(q + 1) * BL] if d == 0
                else xTt[:, ::-1, q * BL:(q + 1) * BL])
        hp = np.asarray(
            inputs[f"h_prev_{'forward' if d == 0 else 'backward'}"])
        h0p = np.ascontiguousarray(
            hp[q * BL:(q + 1) * BL].T.reshape(NKH, 128, BL)
            .transpose(1, 0, 2)).reshape(128, NKH * BL).astype(np.float32)
        in_maps.append({
            "xt": np.ascontiguousarray(view).reshape(I, T * BL),
            "wh": np.ascontiguousarray(whp),
            "wx": np.ascontiguousarray(wxp),
            "bias": biasp, "h0": h0p,
        })

    trace = bool(int(os.environ.get("BIDGRU_TRACE", "0")))
    res = run_bass_kernel_spmd(nc, in_maps, core_ids=list(range(NCORES)),
                               trace=trace)
    if res.exec_time_ns:
        LAST_EXEC_NS = res.exec_time_ns

    out = np.empty((B, T, 2 * H), dtype=np.float32)
    for c in range(NCORES):
        d, q = c // 4, c % 4
        ho = res.results[c]["hout"].reshape(NTB, 128, TB, NM, BL)
        # ho[tb, p, t, m, b] = h[b, tb*TB+t, m*128+p]
        hv = ho.transpose(4, 0, 2, 3, 1).reshape(BL, T, H)
        if d == 1:
            hv = hv[:, ::-1, :]
        out[q * BL:(q + 1) * BL, :, d * H:(d + 1) * H] = hv
    return out


if __name__ == "__main__":
    sys.path.insert(0, "/root/problem")
    build()
    print("build ok")
